# revision 2
# baseline (speedup 1.0000x reference)
"""Trainium2 Bass kernel for JetMoE MoE layer (nn_JetMoeMoE).

Host-routed sparse expert-parallel across 8 NeuronCores:
  - the router (logits + exact top-2 + softmax gates) runs on the host in
    f32/f64 — it is 0.2% of the model FLOPs and determines the dispatch
  - core e receives only the tokens routed to expert e (padded to a fixed
    capacity C, a multiple of 128), plus expert e's weights in bf16
  - on device, core e computes h = x @ wi.T, glu = silu(a)*b, o = glu @ wo.T
    for its C tokens and writes the gate-weighted rows [C, D] f32
  - the host scatter-adds each expert's rows back into the [T, D] output
    (each token appears in exactly two expert lists) and adds the bias

Per-core device work is C*(2H*D + D*H) MACs (~29 GMAC at C=1152) vs the
dense-per-core 103 GMAC of an all-tokens-per-expert layout, and expert
weights (50 MB bf16) stream from HBM exactly once per launch.

Shapes (hardcoded): D=2048, H=4096, E=8, top-2. T = B*L from the input.
"""

import sys

sys.path.insert(0, "/opt/trn_rl_repo")

import math

import numpy as np
import ml_dtypes

import concourse.bass as bass
import concourse.mybir as mybir
import concourse.tile as tile
from concourse import bacc
from concourse.bass_utils import run_bass_kernel_spmd

F32 = mybir.dt.float32
BF16 = mybir.dt.bfloat16
OP = mybir.AluOpType
ACTF = mybir.ActivationFunctionType

P = 128
D = 2048
H = 4096
E = 8
NK1 = D // P      # 16 contraction tiles for the up-proj
NJ = H // P       # 32 GLU feature tiles (tile j pairs with j+NJ)
NK2 = H // P      # 32 contraction tiles for the down-proj
ND = D // 512     # 4 output d-tiles of 512
CMAX = 1280       # max tokens per core per launch (SBUF-resident glu)

BF = ml_dtypes.bfloat16


def _chunks(c):
    return [(s, min(512, c - s)) for s in range(0, c, 512)]


def _emit_moe(tc, xg, g, wi, wo, out, C):
    nc = tc.nc
    nts = C // P
    with (
        tc.tile_pool(name="xgp", bufs=1) as xgp,
        tc.tile_pool(name="gp", bufs=1) as gp,
        tc.tile_pool(name="glup", bufs=1) as glup,
        tc.tile_pool(name="wip", bufs=2) as wip,
        tc.tile_pool(name="wop", bufs=4) as wop,
        tc.tile_pool(name="sap", bufs=2) as sap,
        tc.tile_pool(name="obp", bufs=3) as obp,
        tc.tile_pool(name="ps_h", bufs=2, space="PSUM") as ps_h,
        tc.tile_pool(name="ps_o", bufs=2, space="PSUM") as ps_o,
    ):
        xg_t = xgp.tile([P, NK1, C], BF16)
        for c0, cn in _chunks(C):
            nc.sync.dma_start(xg_t[:, :, c0:c0 + cn], xg[:, :, c0:c0 + cn])
        g_t = gp.tile([P, nts], F32)
        nc.sync.dma_start(g_t[:], g[:])

        # ---- phase 1: hT = wiT.T @ xT per feature tile, GLU -> glu [H, C] bf16
        glu_t = glup.tile([P, NJ, C], BF16)
        for j in range(NJ):
            wia = wip.tile([P, NK1, P], BF16)
            nc.sync.dma_start(wia[:], wi[j])
            wib = wip.tile([P, NK1, P], BF16)
            nc.sync.dma_start(wib[:], wi[j + NJ])
            for c0, cn in _chunks(C):
                pa = ps_h.tile([P, cn], F32, tag="pa")
                for k in range(NK1):
                    nc.tensor.matmul(
                        pa[:], wia[:, k, :], xg_t[:, k, c0:c0 + cn],
                        start=(k == 0), stop=(k == NK1 - 1),
                    )
                pb = ps_h.tile([P, cn], F32, tag="pb")
                for k in range(NK1):
                    nc.tensor.matmul(
                        pb[:], wib[:, k, :], xg_t[:, k, c0:c0 + cn],
                        start=(k == 0), stop=(k == NK1 - 1),
                    )
                sa = sap.tile([P, cn], F32, tag="sa")
                nc.scalar.activation(sa[:], pa[:], ACTF.Silu)
                nc.vector.tensor_mul(glu_t[:, j, c0:c0 + cn], sa[:], pb[:])

        # ---- phase 2: o = gluT.T @ woT per (token, d) tile, gate, store
        for n in range(ND):
            woh0 = wop.tile([P, NK2 // 2, 512], BF16, tag="wo")
            nc.sync.dma_start(woh0[:], wo[n, :, 0:NK2 // 2, :])
            woh1 = wop.tile([P, NK2 // 2, 512], BF16, tag="wo")
            nc.sync.dma_start(woh1[:], wo[n, :, NK2 // 2:NK2, :])
            for ts in range(nts):
                po = ps_o.tile([P, 512], F32)
                for k in range(NK2):
                    wo_c = woh0 if k < NK2 // 2 else woh1
                    nc.tensor.matmul(
                        po[:],
                        glu_t[:, k, ts * P:(ts + 1) * P],
                        wo_c[:, k % (NK2 // 2), :],
                        start=(k == 0), stop=(k == NK2 - 1),
                    )
                ob = obp.tile([P, 512], F32)
                nc.vector.tensor_scalar(
                    ob[:], po[:], g_t[:, ts:ts + 1], None, OP.mult
                )
                nc.sync.dma_start(
                    out[ts * P:(ts + 1) * P, n * 512:(n + 1) * 512], ob[:]
                )


_NC_CACHE = {}


def _get_nc(C):
    if C in _NC_CACHE:
        return _NC_CACHE[C]
    nc = bacc.Bacc("TRN2", target_bir_lowering=False, debug=False, num_devices=8)
    xg = nc.dram_tensor("xg", [P, NK1, C], BF16, kind="ExternalInput")
    g = nc.dram_tensor("g", [P, C // P], F32, kind="ExternalInput")
    wi = nc.dram_tensor("wi", [2 * NJ, P, NK1, P], BF16, kind="ExternalInput")
    wo = nc.dram_tensor("wo", [ND, P, NK2, 512], BF16, kind="ExternalInput")
    out = nc.dram_tensor("out", [C, D], F32, kind="ExternalOutput")
    with tile.TileContext(nc) as tc:
        _emit_moe(tc, xg.ap(), g.ap(), wi.ap(), wo.ap(), out.ap(), C)
    nc.compile()
    _NC_CACHE[C] = nc
    return nc


def _route(x, router_weight):
    """Exact top-2 routing + softmax gates on the host (f32 logits)."""
    logits = (x @ np.asarray(router_weight, np.float32).T).astype(np.float32)
    part = np.argpartition(-logits, 2, axis=1)[:, :2]
    lv = np.take_along_axis(logits, part, 1)
    order = np.argsort(-lv, axis=1, kind="stable")
    top_idx = np.take_along_axis(part, order, 1)          # [T, 2]
    top_val = np.take_along_axis(lv, order, 1).astype(np.float64)
    g1 = 1.0 / (1.0 + np.exp(-(top_val[:, 0] - top_val[:, 1])))
    gates = np.stack([g1, 1.0 - g1], 1).astype(np.float32)  # [T, 2]
    return top_idx, gates


def _weight_maps(w_in, w_out):
    """Per-expert weight layouts (device-tile order, bf16)."""
    maps = []
    for e in range(E):
        wi_r = np.ascontiguousarray(
            np.asarray(w_in[e], np.float32)
            .reshape(2 * NJ, P, NK1, P).transpose(0, 3, 2, 1).astype(BF))
        wo_r = np.ascontiguousarray(
            np.asarray(w_out[e], np.float32)
            .reshape(ND, 512, NK2, P).transpose(0, 3, 2, 1).astype(BF))
        maps.append((wi_r, wo_r))
    return maps


def _prepare(layer_input, router_weight, w_in, w_out, bias):
    """Host routing + sharding. Returns (nc, in_maps list-of-runs, combine)."""
    x = np.ascontiguousarray(np.asarray(layer_input, np.float32).reshape(-1, D))
    T = x.shape[0]
    top_idx, gates = _route(x, router_weight)

    idxs, gvals = [], []
    for e in range(E):
        mask = top_idx == e                                # [T, 2]
        rows = np.nonzero(mask.any(1))[0]
        idxs.append(rows)
        gvals.append(gates[mask])
    max_load = max(len(r) for r in idxs)

    runs = max(1, math.ceil(max_load / CMAX))
    cap = max(1, math.ceil(max_load / runs))
    C = max(P, ((cap + P - 1) // P) * P)
    nc = _get_nc(C)
    wmaps = _weight_maps(w_in, w_out)

    run_in_maps = []
    run_meta = []
    for r in range(runs):
        in_maps = []
        meta = []
        for e in range(E):
            rows = idxs[e][r::runs]
            ge = gvals[e][r::runs]
            load = len(rows)
            xbuf = np.zeros((C, NK1, P), np.float32)
            xbuf[:load] = x[rows].reshape(load, NK1, P)
            xg = np.ascontiguousarray(xbuf.transpose(2, 1, 0)).astype(BF)
            gpad = np.zeros((C,), np.float32)
            gpad[:load] = ge
            g_r = np.ascontiguousarray(gpad.reshape(C // P, P).T)
            in_maps.append({
                "xg": xg, "g": g_r, "wi": wmaps[e][0], "wo": wmaps[e][1],
            })
            meta.append((rows, load))
        run_in_maps.append(in_maps)
        run_meta.append(meta)

    def combine(run_results):
        acc = np.zeros((T, D), np.float32)
        for meta, results in zip(run_meta, run_results):
            for e in range(E):
                rows, load = meta[e]
                if load:
                    acc[rows] += results[e]["out"][:load]
        acc += np.asarray(bias, np.float32)
        return acc

    return nc, run_in_maps, combine


def kernel(layer_input, router_weight, w_in, w_out, bias):
    B, L, _ = np.asarray(layer_input).shape
    nc, run_in_maps, combine = _prepare(
        layer_input, router_weight, w_in, w_out, bias)
    run_results = []
    for in_maps in run_in_maps:
        res = run_bass_kernel_spmd(nc, in_maps, core_ids=list(range(8)))
        run_results.append(res.results)
    return combine(run_results).reshape(B, L, D)


# revision 13
# speedup vs baseline: 141.5938x; 141.5938x over previous
"""Trainium2 Bass kernel for JetMoE MoE layer (nn_JetMoeMoE).

Host-routed sparse expert-parallel across 8 NeuronCores:
  - the router (logits + exact top-2 + softmax gates) runs on the host in
    f32/f64 — it is 0.2% of the model FLOPs and determines the dispatch
  - core e receives only the tokens routed to expert e (padded to a fixed
    capacity C, a multiple of 128), plus expert e's weights in bf16
  - on device, core e computes h = x @ wi.T, glu = silu(a)*b, o = glu @ wo.T
    for its C tokens and writes the gate-weighted rows [C, D] f32
  - the host scatter-adds each expert's rows back into the [T, D] output
    (each token appears in exactly two expert lists) and adds the bias

Per-core device work is C*(2H*D + D*H) MACs (~29 GMAC at C=1152) vs the
dense-per-core 103 GMAC of an all-tokens-per-expert layout, and expert
weights (50 MB bf16) stream from HBM exactly once per launch.

Shapes (hardcoded): D=2048, H=4096, E=8, top-2. T = B*L from the input.
"""

import sys

sys.path.insert(0, "/opt/trn_rl_repo")

import math

import numpy as np
import ml_dtypes

import concourse.bass as bass
import concourse.mybir as mybir
import concourse.tile as tile
from concourse import bacc
from concourse.bass_utils import run_bass_kernel_spmd

F32 = mybir.dt.float32
BF16 = mybir.dt.bfloat16
OP = mybir.AluOpType
ACTF = mybir.ActivationFunctionType

P = 128
D = 2048
H = 4096
E = 8
NK1 = D // P      # 16 contraction tiles for the up-proj
NJ = H // P       # 32 GLU feature tiles (tile j pairs with j+NJ)
NK2 = H // P      # 32 contraction tiles for the down-proj
ND = D // 512     # 4 output d-tiles of 512
CMAX = 1280       # max tokens per core per launch (SBUF-resident glu)

BF = ml_dtypes.bfloat16


def _chunks(c):
    return [(s, min(512, c - s)) for s in range(0, c, 512)]


def _emit_moe(tc, xg, g, wi, wo, out, C, reps=1):
    nc = tc.nc
    nts = (C + P - 1) // P     # phase-2 token tiles; last may be 64 rows
    with (
        tc.tile_pool(name="xgp", bufs=1) as xgp,
        tc.tile_pool(name="gp", bufs=1) as gp,
        tc.tile_pool(name="glup", bufs=1) as glup,
        tc.tile_pool(name="wip", bufs=2) as wip,
        tc.tile_pool(name="wop", bufs=4) as wop,
        tc.tile_pool(name="sap", bufs=2) as sap,
        tc.tile_pool(name="obp", bufs=3) as obp,
        tc.tile_pool(name="ps_h", bufs=2, space="PSUM") as ps_h,
        tc.tile_pool(name="ps_o", bufs=2, space="PSUM") as ps_o,
    ):
        # reps>1 repeats the whole identical pass (timing knob only: the
        # reps-marginal is the exact HW time of one full pass)
        for rep in range(reps):
          xg_t = xgp.tile([P, NK1, C], BF16)
          for c0, cn in _chunks(C):
              nc.sync.dma_start(xg_t[:, :, c0:c0 + cn], xg[:, :, c0:c0 + cn])
          g_t = gp.tile([P, nts], F32)
          nc.sync.dma_start(g_t[:], g[:])

          # ---- phase 1: hT = wiT.T @ xT per feature tile, GLU -> glu bf16
          glu_t = glup.tile([P, NJ, C], BF16)
          for j in range(NJ):
            wia = wip.tile([P, NK1, P], BF16)
            nc.sync.dma_start(wia[:], wi[j])
            wib = wip.tile([P, NK1, P], BF16)
            nc.sync.dma_start(wib[:], wi[j + NJ])
            for c0, cn in _chunks(C):
                pa = ps_h.tile([P, cn], F32, tag="pa")
                for k in range(NK1):
                    nc.tensor.matmul(
                        pa[:], wia[:, k, :], xg_t[:, k, c0:c0 + cn],
                        start=(k == 0), stop=(k == NK1 - 1),
                    )
                pb = ps_h.tile([P, cn], F32, tag="pb")
                for k in range(NK1):
                    nc.tensor.matmul(
                        pb[:], wib[:, k, :], xg_t[:, k, c0:c0 + cn],
                        start=(k == 0), stop=(k == NK1 - 1),
                    )
                sa = sap.tile([P, cn], F32, tag="sa")
                nc.scalar.activation(sa[:], pa[:], ACTF.Silu)
                nc.vector.tensor_mul(glu_t[:, j, c0:c0 + cn], sa[:], pb[:])

          # ---- phase 2: o = gluT.T @ woT per (token, d) tile, gate, store
          for n in range(ND):
            woh0 = wop.tile([P, NK2 // 2, 512], BF16, tag="wo")
            nc.sync.dma_start(woh0[:], wo[n, :, 0:NK2 // 2, :])
            woh1 = wop.tile([P, NK2 // 2, 512], BF16, tag="wo")
            nc.sync.dma_start(woh1[:], wo[n, :, NK2 // 2:NK2, :])
            for ts in range(nts):
                tp = min(P, C - ts * P)    # 128, or 64 on the tail tile
                po = ps_o.tile([tp, 512], F32, tag="po")
                for k in range(NK2):
                    wo_c = woh0 if k < NK2 // 2 else woh1
                    nc.tensor.matmul(
                        po[:],
                        glu_t[:, k, ts * P:ts * P + tp],
                        wo_c[:, k % (NK2 // 2), :],
                        start=(k == 0), stop=(k == NK2 - 1),
                    )
                ob = obp.tile([tp, 512], F32, tag="ob")
                nc.vector.tensor_scalar(
                    ob[:], po[:], g_t[0:tp, ts:ts + 1], None, OP.mult
                )
                nc.sync.dma_start(
                    out[ts * P:ts * P + tp, n * 512:(n + 1) * 512], ob[:]
                )


_NC_CACHE = {}


def _get_nc(C, reps=1):
    key = (C, reps)
    if key in _NC_CACHE:
        return _NC_CACHE[key]
    nc = bacc.Bacc("TRN2", target_bir_lowering=False, debug=False, num_devices=8)
    xg = nc.dram_tensor("xg", [P, NK1, C], BF16, kind="ExternalInput")
    g = nc.dram_tensor("g", [P, (C + P - 1) // P], F32, kind="ExternalInput")
    wi = nc.dram_tensor("wi", [2 * NJ, P, NK1, P], BF16, kind="ExternalInput")
    wo = nc.dram_tensor("wo", [ND, P, NK2, 512], BF16, kind="ExternalInput")
    out = nc.dram_tensor("out", [C, D], F32, kind="ExternalOutput")
    with tile.TileContext(nc) as tc:
        _emit_moe(tc, xg.ap(), g.ap(), wi.ap(), wo.ap(), out.ap(), C, reps=reps)
    nc.compile()
    _NC_CACHE[key] = nc
    return nc


def _route(x, router_weight):
    """Exact top-2 routing + softmax gates on the host (f32 logits)."""
    logits = (x @ np.asarray(router_weight, np.float32).T).astype(np.float32)
    part = np.argpartition(-logits, 2, axis=1)[:, :2]
    lv = np.take_along_axis(logits, part, 1)
    order = np.argsort(-lv, axis=1, kind="stable")
    top_idx = np.take_along_axis(part, order, 1)          # [T, 2]
    top_val = np.take_along_axis(lv, order, 1).astype(np.float64)
    g1 = 1.0 / (1.0 + np.exp(-(top_val[:, 0] - top_val[:, 1])))
    gates = np.stack([g1, 1.0 - g1], 1).astype(np.float32)  # [T, 2]
    return top_idx, gates


def _weight_maps(w_in, w_out):
    """Per-expert weight layouts (device-tile order, bf16)."""
    maps = []
    for e in range(E):
        wi_r = np.ascontiguousarray(
            np.asarray(w_in[e], np.float32)
            .reshape(2 * NJ, P, NK1, P).transpose(0, 3, 2, 1).astype(BF))
        wo_r = np.ascontiguousarray(
            np.asarray(w_out[e], np.float32)
            .reshape(ND, 512, NK2, P).transpose(0, 3, 2, 1).astype(BF))
        maps.append((wi_r, wo_r))
    return maps


def _prepare(layer_input, router_weight, w_in, w_out, bias):
    """Host routing + sharding. Returns (nc, in_maps list-of-runs, combine)."""
    x = np.ascontiguousarray(np.asarray(layer_input, np.float32).reshape(-1, D))
    T = x.shape[0]
    top_idx, gates = _route(x, router_weight)

    idxs, gvals = [], []
    for e in range(E):
        mask = top_idx == e                                # [T, 2]
        rows = np.nonzero(mask.any(1))[0]
        idxs.append(rows)
        gvals.append(gates[mask])
    max_load = max(len(r) for r in idxs)

    runs = max(1, math.ceil(max_load / CMAX))
    cap = max(1, math.ceil(max_load / runs))
    # capacity rounds to 64 (phase-2 handles a 64-row tail tile); the gate
    # array stays 128-padded for its [P, nts] transposed layout
    C = max(P, ((cap + 63) // 64) * 64)
    nc = _get_nc(C)
    wmaps = _weight_maps(w_in, w_out)

    run_in_maps = []
    run_meta = []
    for r in range(runs):
        in_maps = []
        meta = []
        for e in range(E):
            rows = idxs[e][r::runs]
            ge = gvals[e][r::runs]
            load = len(rows)
            xbuf = np.zeros((C, NK1, P), np.float32)
            xbuf[:load] = x[rows].reshape(load, NK1, P)
            xg = np.ascontiguousarray(xbuf.transpose(2, 1, 0)).astype(BF)
            G = ((C + P - 1) // P) * P
            gpad = np.zeros((G,), np.float32)
            gpad[:load] = ge
            g_r = np.ascontiguousarray(gpad.reshape(G // P, P).T)
            in_maps.append({
                "xg": xg, "g": g_r, "wi": wmaps[e][0], "wo": wmaps[e][1],
            })
            meta.append((rows, load))
        run_in_maps.append(in_maps)
        run_meta.append(meta)

    def combine(run_results):
        acc = np.zeros((T, D), np.float32)
        for meta, results in zip(run_meta, run_results):
            for e in range(E):
                rows, load = meta[e]
                if load:
                    acc[rows] += results[e]["out"][:load]
        acc += np.asarray(bias, np.float32)
        return acc

    return nc, run_in_maps, combine


def kernel(layer_input, router_weight, w_in, w_out, bias):
    B, L, _ = np.asarray(layer_input).shape
    nc, run_in_maps, combine = _prepare(
        layer_input, router_weight, w_in, w_out, bias)
    run_results = []
    for in_maps in run_in_maps:
        res = run_bass_kernel_spmd(nc, in_maps, core_ids=list(range(8)))
        run_results.append(res.results)
    return combine(run_results).reshape(B, L, D)
